# revision 10
# baseline (speedup 1.0000x reference)
"""3-level db4 periodic DWT (DWT1DForward) for Trainium2, 8 NeuronCores.

Strategy:
  - Shard batch dim: 16 batches -> 2 per core; each core handles 128
    (batch, channel) lanes x 65536 samples.
  - All 3 levels are fused into one pass: each output sample of
    lo3/hi3/hi2/hi1 is a direct FIR of x (composite filters, 50 taps max,
    circular). Per 128-sample block of x, the 128 outputs that fall in it
    (64 hi1 + 32 hi2 + 16 hi3 + 16 lo3) are a banded matrix multiply of
    the block +/- 32-sample halo.
  - On-chip pipeline per stripe of 32 blocks:
      DMA in (lane-major, contiguous) ->
      PE transpose 128x128 tiles (exact, fp32) -> position-major ->
      PE banded matmuls in fp32r (full speed at free dim 512) ->
      PE transpose back -> DMA out (contiguous per lane).
"""
import sys

sys.path.insert(0, "/opt/trn_rl_repo")

import numpy as np

import concourse.bass as bass
import concourse.bacc as bacc
import concourse.mybir as mybir
from concourse.tile import TileContext
from concourse import bass_utils

F32 = mybir.dt.float32
F32R = mybir.dt.float32r

# Problem constants (hardcoded per harness contract)
B, C, N = 16, 64, 65536
NCORES = 8
LANES = 128            # (B/NCORES) * C = 2*64 lanes per core
NBLK = N // 128        # 512 blocks per lane
SB = 32                # blocks per stripe
NSTRIPE = NBLK // SB   # 16 stripes

# bisection flags (debug only)
EN_WM = True
EN_WP = True
EN_ACT = True


def _build_filters(h0, h1):
    h = h0.astype(np.float64)
    g = h1.astype(np.float64)
    L = 8
    f2h = np.zeros(22)
    f2g = np.zeros(22)
    for s in range(L):
        for t in range(L):
            f2h[2 * s + t] += h[s] * h[t]
            f2g[2 * s + t] += g[s] * h[t]
    f3h = np.zeros(50)
    f3g = np.zeros(50)
    for s in range(L):
        for u in range(22):
            f3h[4 * s + u] += h[s] * f2h[u]
            f3g[4 * s + u] += g[s] * f2h[u]
    return f2g, f3g, f3h


def build_W(h0, h1):
    """Wbig[(q+128), c], q in [-128, 256): coeff of x[128b+q] in channel c of
    block b. Channels: 0..63 hi1, 64..95 hi2, 96..111 hi3, 112..127 lo3."""
    f2g, f3g, f3h = _build_filters(h0, h1)
    g = h1.astype(np.float64)
    Wbig = np.zeros((384, 128))
    for c in range(128):
        if c < 64:
            o, d, taps, stride = c, -3, g, 2
        elif c < 96:
            o, d, taps, stride = c - 64, -9, f2g, 4
        elif c < 112:
            o, d, taps, stride = c - 96, -21, f3g, 8
        else:
            o, d, taps, stride = c - 112, -21, f3h, 8
        for t in range(len(taps)):
            q = stride * o + d + t
            Wbig[q + 128, c] += taps[t]
    return Wbig.astype(np.float32)


def build_nc():
    nc = bacc.Bacc(target_bir_lowering=False)
    x = nc.dram_tensor("x", [LANES, N], F32, kind="ExternalInput")
    # W input: rows 0..31 = Wp (q 128..159), rows 32..159 unused pad,
    # packed as one [128, 384] tensor: col-blocks [W0 | Wm_at_rows96 | Wp_at_rows0]
    wmat = nc.dram_tensor("wmat", [128, 384], F32, kind="ExternalInput")
    ident = nc.dram_tensor("ident", [128, 128], F32, kind="ExternalInput")
    hi1 = nc.dram_tensor("hi1", [LANES, N // 2], F32, kind="ExternalOutput")
    hi2 = nc.dram_tensor("hi2", [LANES, N // 4], F32, kind="ExternalOutput")
    hi3 = nc.dram_tensor("hi3", [LANES, N // 8], F32, kind="ExternalOutput")
    lo3 = nc.dram_tensor("lo3", [LANES, N // 8], F32, kind="ExternalOutput")

    SBC = SB * 128           # stripe columns (conv region)
    XC = (SB + 2) * 128      # stripe + 2 halo blocks

    with TileContext(nc) as tc:
        with (
            tc.tile_pool(name="consts", bufs=1) as cpool,
            tc.tile_pool(name="xin", bufs=2) as xin_pool,
            tc.tile_pool(name="xt", bufs=2) as xt_pool,
            tc.tile_pool(name="conv", bufs=2) as conv_pool,
            tc.tile_pool(name="outp", bufs=2) as out_pool,
            tc.tile_pool(name="pst", bufs=2, space="PSUM") as pst_pool,
            tc.tile_pool(name="psc", bufs=2, space="PSUM") as psc_pool,
            tc.tile_pool(name="pso", bufs=2, space="PSUM") as pso_pool,
        ):
            wt = cpool.tile([128, 384], F32)
            wtr = cpool.tile([128, 384], F32R)
            it = cpool.tile([128, 128], F32)
            nc.sync.dma_start(wt[:], wmat[:])
            nc.sync.dma_start(it[:], ident[:])
            nc.vector.tensor_copy(wtr[:], wt[:])
            w0 = wtr[:, 0:128]
            wm = wtr[:, 128:256]   # full 128 rows: q=-128..-1 (only tail nonzero)
            wp = wtr[:, 256:384]   # full 128 rows: q=128..255 (only head nonzero)

            for s in range(NSTRIPE):
                b0 = s * SB
                x_sb = xin_pool.tile([128, XC], F32)
                # halo-inclusive load: blocks b0-1 .. b0+SB (circular)
                if s == 0:
                    nc.sync.dma_start(x_sb[:, 0:128], x[:, (NBLK - 1) * 128:])
                    nc.sync.dma_start(
                        x_sb[:, 128:XC], x[:, 0:(SB + 1) * 128]
                    )
                elif s == NSTRIPE - 1:
                    nc.sync.dma_start(
                        x_sb[:, 0:XC - 128],
                        x[:, (b0 - 1) * 128:(b0 + SB) * 128],
                    )
                    nc.sync.dma_start(x_sb[:, XC - 128:XC], x[:, 0:128])
                else:
                    nc.sync.dma_start(
                        x_sb[:], x[:, (b0 - 1) * 128:(b0 + SB + 1) * 128]
                    )

                # T-in: transpose each 128x128 tile -> xt (position-major)
                xt = xt_pool.tile([128, XC], F32R)
                n_t = SB + 2
                for j0 in range(0, n_t, 4):
                    jn = min(4, n_t - j0)
                    ps = pst_pool.tile([128, 512], F32)
                    for j in range(j0, j0 + jn):
                        nc.tensor.matmul(
                            ps[:, (j - j0) * 128:(j - j0 + 1) * 128],
                            x_sb[:, j * 128:(j + 1) * 128],
                            it[:],
                            is_transpose=True,
                            start=(j == j0),
                            stop=(j == j0 + jn - 1),
                        )
                    if (j0 // 4) % 2 == 0 or not EN_ACT:
                        nc.vector.tensor_copy(
                            xt[:, j0 * 128:(j0 + jn) * 128], ps[:, 0:jn * 128])
                    else:
                        nc.scalar.copy(
                            xt[:, j0 * 128:(j0 + jn) * 128], ps[:, 0:jn * 128])

                # conv: banded matmuls, fp32r. xt col 128.. is block b0.
                conv_sb = conv_pool.tile([128, SBC], F32)
                xtr = xt[:]
                for gi in range(SBC // 512):
                    pc = psc_pool.tile([128, 512], F32)
                    base = 128 + gi * 512
                    nc.tensor.matmul(
                        pc[:], w0, xtr[:, base:base + 512],
                        start=True, stop=not (EN_WM or EN_WP),
                    )
                    if EN_WM:
                        nc.tensor.matmul(
                            pc[:], wm, xtr[:, base - 128:base + 384],
                            start=False, stop=not EN_WP,
                        )
                    if EN_WP:
                        nc.tensor.matmul(
                            pc[:], wp, xtr[:, base + 128:base + 640],
                            start=False, stop=True,
                        )
                    if gi % 2 == 0 or not EN_ACT:
                        nc.vector.tensor_copy(
                            conv_sb[:, gi * 512:(gi + 1) * 512], pc[:])
                    else:
                        nc.scalar.copy(
                            conv_sb[:, gi * 512:(gi + 1) * 512], pc[:])

                # T-out: transpose back to lane-major
                out_sb = out_pool.tile([128, SBC], F32)
                for j0 in range(0, SB, 4):
                    po = pso_pool.tile([128, 512], F32)
                    for j in range(j0, j0 + 4):
                        nc.tensor.matmul(
                            po[:, (j - j0) * 128:(j - j0 + 1) * 128],
                            conv_sb[:, j * 128:(j + 1) * 128],
                            it[:],
                            is_transpose=True,
                            start=(j == j0),
                            stop=(j == j0 + 3),
                        )
                    if (j0 // 4) % 2 == 0 or not EN_ACT:
                        nc.vector.tensor_copy(
                            out_sb[:, j0 * 128:(j0 + 4) * 128], po[:])
                    else:
                        nc.scalar.copy(
                            out_sb[:, j0 * 128:(j0 + 4) * 128], po[:])

                # DMA out: out_sb[l, j*128 + c] ; c 0:64 hi1, 64:96 hi2,
                # 96:112 hi3, 112:128 lo3
                src = out_sb[:].rearrange("p (j c) -> p j c", c=128)
                nc.sync.dma_start(
                    hi1[:, 64 * b0:64 * (b0 + SB)].rearrange(
                        "p (j c) -> p j c", c=64),
                    src[:, :, 0:64],
                )
                nc.sync.dma_start(
                    hi2[:, 32 * b0:32 * (b0 + SB)].rearrange(
                        "p (j c) -> p j c", c=32),
                    src[:, :, 64:96],
                )
                nc.sync.dma_start(
                    hi3[:, 16 * b0:16 * (b0 + SB)].rearrange(
                        "p (j c) -> p j c", c=16),
                    src[:, :, 96:112],
                )
                nc.sync.dma_start(
                    lo3[:, 16 * b0:16 * (b0 + SB)].rearrange(
                        "p (j c) -> p j c", c=16),
                    src[:, :, 112:128],
                )
    nc.finalize()
    return nc


TRACE = False
LAST_RESULT = None

_NC_CACHE = None


def _get_nc():
    global _NC_CACHE
    if _NC_CACHE is None:
        _NC_CACHE = build_nc()
    return _NC_CACHE


def kernel(x, h0, h1, **_ignored):
    x = np.ascontiguousarray(np.asarray(x, dtype=np.float32))
    h0 = np.asarray(h0, dtype=np.float32)
    h1 = np.asarray(h1, dtype=np.float32)
    assert x.shape == (B, C, N)

    Wbig = build_W(h0, h1)  # (384, 128)
    wmat = np.zeros((128, 384), dtype=np.float32)
    wmat[:, 0:128] = Wbig[128:256]    # W0: q 0..127
    wmat[:, 128:256] = Wbig[0:128]    # Wm: q -128..-1 (only last 21 rows nonzero)
    wmat[:, 256:384] = Wbig[256:384]  # Wp: q 128..255 (only first 21 rows nonzero)
    ident = np.eye(128, dtype=np.float32)

    nc = _get_nc()
    bper = B // NCORES
    in_maps = []
    for i in range(NCORES):
        xs = x[i * bper:(i + 1) * bper].reshape(LANES, N)
        in_maps.append({
            "x": np.ascontiguousarray(xs),
            "wmat": wmat,
            "ident": ident,
        })
    kwargs = {}
    if TRACE:
        kwargs = dict(trace=True, trace_cores=[0])
    res = bass_utils.run_bass_kernel_spmd(
        nc, in_maps, core_ids=list(range(NCORES)), **kwargs
    )
    global LAST_RESULT
    LAST_RESULT = res
    lo3 = np.concatenate(
        [r["lo3"].reshape(bper, C, N // 8) for r in res.results], axis=0)
    hi1 = np.concatenate(
        [r["hi1"].reshape(bper, C, N // 2) for r in res.results], axis=0)
    hi2 = np.concatenate(
        [r["hi2"].reshape(bper, C, N // 4) for r in res.results], axis=0)
    hi3 = np.concatenate(
        [r["hi3"].reshape(bper, C, N // 8) for r in res.results], axis=0)
    return lo3, hi1, hi2, hi3


# revision 12
# speedup vs baseline: 1.7696x; 1.7696x over previous
"""3-level db4 periodic DWT (DWT1DForward) for Trainium2, 8 NeuronCores.

Strategy:
  - Shard batch dim: 16 batches -> 2 per core; each core handles 128
    (batch, channel) lanes x 65536 samples.
  - All 3 levels are fused into one pass: each output sample of
    lo3/hi3/hi2/hi1 is a direct FIR of x (composite filters, 50 taps max,
    circular). Per 128-sample block of x, 128 outputs (64 hi1 + 32 hi2 +
    16 hi3 + 16 lo3) are a banded matrix multiply of the block plus a
    right-side halo (one-sided via output index shift; the host un-shifts
    with np.roll during unsharding).
  - On-chip pipeline per stripe of 32 blocks:
      DMA in (lane-major, contiguous) ->
      PE transpose 128x128 tiles (exact, fp32) -> position-major ->
      2 banded matmuls in fp32r (full speed at free dim 512) ->
      PE transpose back -> channel-grouped staging -> contiguous DMA out.
"""
import sys

sys.path.insert(0, "/opt/trn_rl_repo")

import numpy as np

import concourse.bass as bass
import concourse.bacc as bacc
import concourse.mybir as mybir
from concourse.tile import TileContext
from concourse import bass_utils

F32 = mybir.dt.float32
F32R = mybir.dt.float32r

# Problem constants (hardcoded per harness contract)
B, C, N = 16, 64, 65536
NCORES = 8
LANES = 128            # (B/NCORES) * C = 2*64 lanes per core
NBLK = N // 128        # 512 blocks per lane
SB = 32                # blocks per stripe
NSTRIPE = NBLK // SB   # 16 stripes

TRACE = False
LAST_RESULT = None
_NC_CACHE = None


def _build_filters(h0, h1):
    h = h0.astype(np.float64)
    g = h1.astype(np.float64)
    L = 8
    f2h = np.zeros(22)
    f2g = np.zeros(22)
    for s in range(L):
        for t in range(L):
            f2h[2 * s + t] += h[s] * h[t]
            f2g[2 * s + t] += g[s] * h[t]
    f3h = np.zeros(50)
    f3g = np.zeros(50)
    for s in range(L):
        for u in range(22):
            f3h[4 * s + u] += h[s] * f2h[u]
            f3g[4 * s + u] += g[s] * f2h[u]
    return f2g, f3g, f3h


def build_W(h0, h1):
    """Wbig2[q, c] for q in [0, 256): coeff of x[128b + q] in channel c of
    block b, with shifted output indexing:
      c in [0,64):    hi1[64b + 2 + c]   q = 2c + 1 + t,  t in [0,8)
      c in [64,96):   hi2[32b + 3 + c']  q = 4c' + 3 + u, u in [0,22)
      c in [96,112):  hi3[16b + 3 + c']  q = 8c' + 3 + v, v in [0,50)
      c in [112,128): lo3[16b + 3 + c']  q = 8c' + 3 + v
    Host unshifts with np.roll(out, shift) where shift = 2 (hi1) or 3.
    """
    f2g, f3g, f3h = _build_filters(h0, h1)
    g = h1.astype(np.float64)
    Wbig = np.zeros((256, 128))
    for c in range(128):
        if c < 64:
            o, d, taps, stride = c, 1, g, 2
        elif c < 96:
            o, d, taps, stride = c - 64, 3, f2g, 4
        elif c < 112:
            o, d, taps, stride = c - 96, 3, f3g, 8
        else:
            o, d, taps, stride = c - 112, 3, f3h, 8
        for t in range(len(taps)):
            q = stride * o + d + t
            Wbig[q, c] += taps[t]
    return Wbig.astype(np.float32)


def build_nc():
    nc = bacc.Bacc(target_bir_lowering=False)
    x = nc.dram_tensor("x", [LANES, N], F32, kind="ExternalInput")
    # wmat: [128, 256] = [W0 | Wp]; Wp nonzero only in rows 0..44
    wmat = nc.dram_tensor("wmat", [128, 256], F32, kind="ExternalInput")
    ident = nc.dram_tensor("ident", [128, 128], F32, kind="ExternalInput")
    hi1 = nc.dram_tensor("hi1", [LANES, N // 2], F32, kind="ExternalOutput")
    hi2 = nc.dram_tensor("hi2", [LANES, N // 4], F32, kind="ExternalOutput")
    hi3 = nc.dram_tensor("hi3", [LANES, N // 8], F32, kind="ExternalOutput")
    lo3 = nc.dram_tensor("lo3", [LANES, N // 8], F32, kind="ExternalOutput")

    SBC = SB * 128           # stripe columns (conv region)
    XC = (SB + 1) * 128      # stripe + right halo block

    with TileContext(nc) as tc:
        with (
            tc.tile_pool(name="consts", bufs=1) as cpool,
            tc.tile_pool(name="xin", bufs=2) as xin_pool,
            tc.tile_pool(name="xt", bufs=2) as xt_pool,
            tc.tile_pool(name="conv", bufs=2) as conv_pool,
            tc.tile_pool(name="outp", bufs=2) as out_pool,
            tc.tile_pool(name="pst", bufs=2, space="PSUM") as pst_pool,
            tc.tile_pool(name="psc", bufs=2, space="PSUM") as psc_pool,
            tc.tile_pool(name="pso", bufs=2, space="PSUM") as pso_pool,
        ):
            wt = cpool.tile([128, 256], F32)
            wtr = cpool.tile([128, 256], F32R)
            it = cpool.tile([128, 128], F32)
            nc.sync.dma_start(wt[:], wmat[:])
            nc.sync.dma_start(it[:], ident[:])
            nc.vector.tensor_copy(wtr[:], wt[:])
            w0 = wtr[:, 0:128]
            wp = wtr[:, 128:256]

            ecnt = [0]

            def copy(o, i):
                if ecnt[0] % 2 == 0:
                    nc.vector.tensor_copy(o, i)
                else:
                    nc.scalar.copy(o, i)
                ecnt[0] += 1

            for s in range(NSTRIPE):
                b0 = s * SB
                x_sb = xin_pool.tile([128, XC], F32, name=f"xsb{s}", tag="xsb")
                # load blocks b0 .. b0+SB (right halo, circular at the end)
                if s == NSTRIPE - 1:
                    nc.sync.dma_start(
                        x_sb[:, 0:SBC], x[:, b0 * 128:(b0 + SB) * 128])
                    nc.sync.dma_start(x_sb[:, SBC:XC], x[:, 0:128])
                else:
                    nc.sync.dma_start(
                        x_sb[:], x[:, b0 * 128:(b0 + SB + 1) * 128])

                # T-in: transpose each 128x128 tile -> xt (position-major)
                xt = xt_pool.tile([128, XC], F32R, name=f"xt{s}", tag="xt")
                n_t = SB + 1
                for j0 in range(0, n_t, 4):
                    jn = min(4, n_t - j0)
                    ps = pst_pool.tile([128, 512], F32, name=f"ps{s}_{j0}",
                                       tag="pst")
                    for j in range(j0, j0 + jn):
                        nc.tensor.matmul(
                            ps[:, (j - j0) * 128:(j - j0 + 1) * 128],
                            x_sb[:, j * 128:(j + 1) * 128],
                            it[:],
                            is_transpose=True,
                            start=(j == j0),
                            stop=(j == j0 + jn - 1),
                        )
                    copy(xt[:, j0 * 128:(j0 + jn) * 128], ps[:, 0:jn * 128])

                # conv: 2 banded matmuls (cur + right halo), fp32r
                conv_sb = conv_pool.tile([128, SBC], F32, name=f"cv{s}",
                                         tag="conv")
                for gi in range(SBC // 512):
                    pc = psc_pool.tile([128, 512], F32, name=f"pc{s}_{gi}",
                                       tag="psc")
                    base = gi * 512
                    nc.tensor.matmul(
                        pc[:], w0, xt[:, base:base + 512],
                        start=True, stop=False,
                    )
                    nc.tensor.matmul(
                        pc[:], wp, xt[:, base + 128:base + 640],
                        start=False, stop=True,
                    )
                    copy(conv_sb[:, gi * 512:(gi + 1) * 512], pc[:])

                # T-out: transpose back to lane-major, 8 blocks per 2-bank
                # psum tile, then channel-grouped copies into staging tiles
                h1_sb = out_pool.tile([128, SB * 64], F32, name=f"h1s{s}",
                                      tag="h1s")
                h2_sb = out_pool.tile([128, SB * 32], F32, name=f"h2s{s}",
                                      tag="h2s")
                h3_sb = out_pool.tile([128, SB * 16], F32, name=f"h3s{s}",
                                      tag="h3s")
                l3_sb = out_pool.tile([128, SB * 16], F32, name=f"l3s{s}",
                                      tag="l3s")
                for j0 in range(0, SB, 8):
                    po = pso_pool.tile([128, 1024], F32, name=f"po{s}_{j0}",
                                       tag="pso")
                    for j in range(j0, j0 + 8):
                        nc.tensor.matmul(
                            po[:, (j - j0) * 128:(j - j0 + 1) * 128],
                            conv_sb[:, j * 128:(j + 1) * 128],
                            it[:],
                            is_transpose=True,
                            start=((j - j0) % 4 == 0),
                            stop=((j - j0) % 4 == 3),
                        )
                    pv = po[:].rearrange("p (j c) -> p j c", c=128)
                    copy(
                        h1_sb[:, j0 * 64:(j0 + 8) * 64].rearrange(
                            "p (j c) -> p j c", c=64),
                        pv[:, :, 0:64])
                    copy(
                        h2_sb[:, j0 * 32:(j0 + 8) * 32].rearrange(
                            "p (j c) -> p j c", c=32),
                        pv[:, :, 64:96])
                    copy(
                        h3_sb[:, j0 * 16:(j0 + 8) * 16].rearrange(
                            "p (j c) -> p j c", c=16),
                        pv[:, :, 96:112])
                    copy(
                        l3_sb[:, j0 * 16:(j0 + 8) * 16].rearrange(
                            "p (j c) -> p j c", c=16),
                        pv[:, :, 112:128])

                # DMA out: fully contiguous on both sides
                nc.sync.dma_start(hi1[:, 64 * b0:64 * (b0 + SB)], h1_sb[:])
                nc.sync.dma_start(hi2[:, 32 * b0:32 * (b0 + SB)], h2_sb[:])
                nc.sync.dma_start(hi3[:, 16 * b0:16 * (b0 + SB)], h3_sb[:])
                nc.sync.dma_start(lo3[:, 16 * b0:16 * (b0 + SB)], l3_sb[:])
    nc.finalize()
    return nc


def _get_nc():
    global _NC_CACHE
    if _NC_CACHE is None:
        _NC_CACHE = build_nc()
    return _NC_CACHE


def kernel(x, h0, h1, **_ignored):
    x = np.ascontiguousarray(np.asarray(x, dtype=np.float32))
    h0 = np.asarray(h0, dtype=np.float32)
    h1 = np.asarray(h1, dtype=np.float32)
    assert x.shape == (B, C, N)

    Wbig = build_W(h0, h1)  # (256, 128)
    wmat = np.ascontiguousarray(
        np.concatenate([Wbig[0:128], Wbig[128:256]], axis=1),
        dtype=np.float32)
    ident = np.eye(128, dtype=np.float32)

    nc = _get_nc()
    bper = B // NCORES
    in_maps = []
    for i in range(NCORES):
        xs = x[i * bper:(i + 1) * bper].reshape(LANES, N)
        in_maps.append({
            "x": np.ascontiguousarray(xs),
            "wmat": wmat,
            "ident": ident,
        })
    kwargs = {}
    if TRACE:
        kwargs = dict(trace=True, trace_cores=[0])
    res = bass_utils.run_bass_kernel_spmd(
        nc, in_maps, core_ids=list(range(NCORES)), **kwargs
    )
    global LAST_RESULT
    LAST_RESULT = res

    def gather(name, n_out, shift):
        full = np.concatenate(
            [r[name].reshape(bper, C, n_out) for r in res.results], axis=0)
        return np.roll(full, shift, axis=-1)

    lo3 = gather("lo3", N // 8, 3)
    hi1 = gather("hi1", N // 2, 2)
    hi2 = gather("hi2", N // 4, 3)
    hi3 = gather("hi3", N // 8, 3)
    return lo3, hi1, hi2, hi3


# revision 17
# speedup vs baseline: 1.7743x; 1.0026x over previous
"""3-level db4 periodic DWT (DWT1DForward) for Trainium2, 8 NeuronCores.

Strategy:
  - Shard batch dim: 16 batches -> 2 per core; each core handles 128
    (batch, channel) lanes x 65536 samples.
  - All 3 levels are fused into one pass: each output sample of
    lo3/hi3/hi2/hi1 is a direct FIR of x (composite filters, 50 taps max,
    circular). Per 128-sample block of x, 128 outputs (64 hi1 + 32 hi2 +
    16 hi3 + 16 lo3) are a banded matrix multiply of the block plus a
    right-side halo (one-sided via output index shift; the host un-shifts
    with np.roll during unsharding).
  - On-chip pipeline per stripe of 32 blocks:
      DMA in (lane-major, contiguous) ->
      PE transpose 128x128 tiles (exact, fp32) -> position-major ->
      2 banded matmuls in fp32r (full speed at free dim 512) ->
      PE transpose back -> channel-grouped staging -> contiguous DMA out.
"""
import sys

sys.path.insert(0, "/opt/trn_rl_repo")

import numpy as np

import concourse.bass as bass
import concourse.bacc as bacc
import concourse.mybir as mybir
from concourse.tile import TileContext
from concourse import bass_utils

F32 = mybir.dt.float32
F32R = mybir.dt.float32r

# Problem constants (hardcoded per harness contract)
B, C, N = 16, 64, 65536
NCORES = 8
LANES = 128            # (B/NCORES) * C = 2*64 lanes per core
NBLK = N // 128        # 512 blocks per lane
SB = 32                # blocks per stripe
NSTRIPE = NBLK // SB   # 16 stripes

TRACE = False
LAST_RESULT = None
_NC_CACHE = None


def _build_filters(h0, h1):
    h = h0.astype(np.float64)
    g = h1.astype(np.float64)
    L = 8
    f2h = np.zeros(22)
    f2g = np.zeros(22)
    for s in range(L):
        for t in range(L):
            f2h[2 * s + t] += h[s] * h[t]
            f2g[2 * s + t] += g[s] * h[t]
    f3h = np.zeros(50)
    f3g = np.zeros(50)
    for s in range(L):
        for u in range(22):
            f3h[4 * s + u] += h[s] * f2h[u]
            f3g[4 * s + u] += g[s] * f2h[u]
    return f2g, f3g, f3h


def build_W(h0, h1):
    """Wbig2[q, c] for q in [0, 256): coeff of x[128b + q] in channel c of
    block b, with shifted output indexing:
      c in [0,64):    hi1[64b + 2 + c]   q = 2c + 1 + t,  t in [0,8)
      c in [64,96):   hi2[32b + 3 + c']  q = 4c' + 3 + u, u in [0,22)
      c in [96,112):  hi3[16b + 3 + c']  q = 8c' + 3 + v, v in [0,50)
      c in [112,128): lo3[16b + 3 + c']  q = 8c' + 3 + v
    Host unshifts with np.roll(out, shift) where shift = 2 (hi1) or 3.
    """
    f2g, f3g, f3h = _build_filters(h0, h1)
    g = h1.astype(np.float64)
    Wbig = np.zeros((256, 128))
    for c in range(128):
        if c < 64:
            o, d, taps, stride = c, 1, g, 2
        elif c < 96:
            o, d, taps, stride = c - 64, 3, f2g, 4
        elif c < 112:
            o, d, taps, stride = c - 96, 3, f3g, 8
        else:
            o, d, taps, stride = c - 112, 3, f3h, 8
        for t in range(len(taps)):
            q = stride * o + d + t
            Wbig[q, c] += taps[t]
    return Wbig.astype(np.float32)


def build_nc():
    nc = bacc.Bacc(target_bir_lowering=False)
    x = nc.dram_tensor("x", [LANES, N], F32, kind="ExternalInput")
    # wmat: [128, 256] = [W0 | Wp]; Wp nonzero only in rows 0..44
    wmat = nc.dram_tensor("wmat", [128, 256], F32, kind="ExternalInput")
    ident = nc.dram_tensor("ident", [128, 128], F32, kind="ExternalInput")
    hi1 = nc.dram_tensor("hi1", [LANES, N // 2], F32, kind="ExternalOutput")
    hi2 = nc.dram_tensor("hi2", [LANES, N // 4], F32, kind="ExternalOutput")
    hi3 = nc.dram_tensor("hi3", [LANES, N // 8], F32, kind="ExternalOutput")
    lo3 = nc.dram_tensor("lo3", [LANES, N // 8], F32, kind="ExternalOutput")

    SBC = SB * 128           # stripe columns (conv region)
    XC = (SB + 1) * 128      # stripe + right halo block

    with TileContext(nc) as tc:
        with (
            tc.tile_pool(name="consts", bufs=1) as cpool,
            tc.tile_pool(name="xin", bufs=2) as xin_pool,
            tc.tile_pool(name="xt", bufs=2) as xt_pool,
            tc.tile_pool(name="conv", bufs=3) as conv_pool,
            tc.tile_pool(name="outp", bufs=3) as out_pool,
            tc.tile_pool(name="pst", bufs=2, space="PSUM") as pst_pool,
            tc.tile_pool(name="psc", bufs=2, space="PSUM") as psc_pool,
            tc.tile_pool(name="pso", bufs=2, space="PSUM") as pso_pool,
        ):
            wt = cpool.tile([128, 256], F32)
            wtr = cpool.tile([128, 256], F32R)
            it = cpool.tile([128, 128], F32)
            nc.sync.dma_start(wt[:], wmat[:])
            nc.sync.dma_start(it[:], ident[:])
            nc.vector.tensor_copy(wtr[:], wt[:])
            w0 = wtr[:, 0:128]
            wp = wtr[:, 128:256]

            ecnt = [0]

            def copy(o, i):
                if ecnt[0] % 2 == 0:
                    nc.vector.tensor_copy(o, i)
                else:
                    nc.scalar.copy(o, i)
                ecnt[0] += 1

            for s in range(NSTRIPE):
                b0 = s * SB
                x_sb = xin_pool.tile([128, XC], F32, name=f"xsb{s}", tag="xsb")
                # load blocks b0 .. b0+SB (right halo, circular at the end)
                if s == NSTRIPE - 1:
                    nc.sync.dma_start(
                        x_sb[:, 0:SBC], x[:, b0 * 128:(b0 + SB) * 128])
                    nc.sync.dma_start(x_sb[:, SBC:XC], x[:, 0:128])
                else:
                    nc.sync.dma_start(
                        x_sb[:], x[:, b0 * 128:(b0 + SB + 1) * 128])

                # T-in: transpose each 128x128 tile -> xt (position-major)
                xt = xt_pool.tile([128, XC], F32R, name=f"xt{s}", tag="xt")
                n_t = SB + 1
                for j0 in range(0, n_t, 4):
                    jn = min(4, n_t - j0)
                    ps = pst_pool.tile([128, 512], F32, name=f"ps{s}_{j0}",
                                       tag="pst")
                    for j in range(j0, j0 + jn):
                        nc.tensor.matmul(
                            ps[:, (j - j0) * 128:(j - j0 + 1) * 128],
                            x_sb[:, j * 128:(j + 1) * 128],
                            it[:],
                            is_transpose=True,
                            start=(j == j0),
                            stop=(j == j0 + jn - 1),
                        )
                    copy(xt[:, j0 * 128:(j0 + jn) * 128], ps[:, 0:jn * 128])

                # conv: 2 banded matmuls (cur + right halo), fp32r
                conv_sb = conv_pool.tile([128, SBC], F32, name=f"cv{s}",
                                         tag="conv")
                for gi in range(SBC // 512):
                    pc = psc_pool.tile([128, 512], F32, name=f"pc{s}_{gi}",
                                       tag="psc")
                    base = gi * 512
                    nc.tensor.matmul(
                        pc[:], w0, xt[:, base:base + 512],
                        start=True, stop=False,
                    )
                    nc.tensor.matmul(
                        pc[:], wp, xt[:, base + 128:base + 640],
                        start=False, stop=True,
                    )
                    copy(conv_sb[:, gi * 512:(gi + 1) * 512], pc[:])

                # T-out: transpose back to lane-major, 8 blocks per 2-bank
                # psum tile, then channel-grouped copies into staging tiles
                h1_sb = out_pool.tile([128, SB * 64], F32, name=f"h1s{s}",
                                      tag="h1s")
                h2_sb = out_pool.tile([128, SB * 32], F32, name=f"h2s{s}",
                                      tag="h2s")
                h3_sb = out_pool.tile([128, SB * 16], F32, name=f"h3s{s}",
                                      tag="h3s")
                l3_sb = out_pool.tile([128, SB * 16], F32, name=f"l3s{s}",
                                      tag="l3s")
                for j0 in range(0, SB, 8):
                    po = pso_pool.tile([128, 1024], F32, name=f"po{s}_{j0}",
                                       tag="pso")
                    for j in range(j0, j0 + 8):
                        nc.tensor.matmul(
                            po[:, (j - j0) * 128:(j - j0 + 1) * 128],
                            conv_sb[:, j * 128:(j + 1) * 128],
                            it[:],
                            is_transpose=True,
                            start=((j - j0) % 4 == 0),
                            stop=((j - j0) % 4 == 3),
                        )
                    pv = po[:].rearrange("p (j c) -> p j c", c=128)
                    copy(
                        h1_sb[:, j0 * 64:(j0 + 8) * 64].rearrange(
                            "p (j c) -> p j c", c=64),
                        pv[:, :, 0:64])
                    copy(
                        h2_sb[:, j0 * 32:(j0 + 8) * 32].rearrange(
                            "p (j c) -> p j c", c=32),
                        pv[:, :, 64:96])
                    copy(
                        h3_sb[:, j0 * 16:(j0 + 8) * 16].rearrange(
                            "p (j c) -> p j c", c=16),
                        pv[:, :, 96:112])
                    copy(
                        l3_sb[:, j0 * 16:(j0 + 8) * 16].rearrange(
                            "p (j c) -> p j c", c=16),
                        pv[:, :, 112:128])

                # DMA out: fully contiguous on both sides
                nc.sync.dma_start(hi1[:, 64 * b0:64 * (b0 + SB)], h1_sb[:])
                nc.sync.dma_start(hi2[:, 32 * b0:32 * (b0 + SB)], h2_sb[:])
                nc.sync.dma_start(hi3[:, 16 * b0:16 * (b0 + SB)], h3_sb[:])
                nc.sync.dma_start(lo3[:, 16 * b0:16 * (b0 + SB)], l3_sb[:])
    nc.finalize()
    return nc


def _get_nc():
    global _NC_CACHE
    if _NC_CACHE is None:
        _NC_CACHE = build_nc()
    return _NC_CACHE


def kernel(x, h0, h1, **_ignored):
    x = np.ascontiguousarray(np.asarray(x, dtype=np.float32))
    h0 = np.asarray(h0, dtype=np.float32)
    h1 = np.asarray(h1, dtype=np.float32)
    assert x.shape == (B, C, N)

    Wbig = build_W(h0, h1)  # (256, 128)
    wmat = np.ascontiguousarray(
        np.concatenate([Wbig[0:128], Wbig[128:256]], axis=1),
        dtype=np.float32)
    ident = np.eye(128, dtype=np.float32)

    nc = _get_nc()
    bper = B // NCORES
    in_maps = []
    for i in range(NCORES):
        xs = x[i * bper:(i + 1) * bper].reshape(LANES, N)
        in_maps.append({
            "x": np.ascontiguousarray(xs),
            "wmat": wmat,
            "ident": ident,
        })
    kwargs = {}
    if TRACE:
        kwargs = dict(trace=True, trace_cores=[0])
    res = bass_utils.run_bass_kernel_spmd(
        nc, in_maps, core_ids=list(range(NCORES)), **kwargs
    )
    global LAST_RESULT
    LAST_RESULT = res

    def gather(name, n_out, shift):
        full = np.concatenate(
            [r[name].reshape(bper, C, n_out) for r in res.results], axis=0)
        return np.roll(full, shift, axis=-1)

    lo3 = gather("lo3", N // 8, 3)
    hi1 = gather("hi1", N // 2, 2)
    hi2 = gather("hi2", N // 4, 3)
    hi3 = gather("hi3", N // 8, 3)
    return lo3, hi1, hi2, hi3


# revision 18
# speedup vs baseline: 1.8080x; 1.0190x over previous
"""3-level db4 periodic DWT (DWT1DForward) for Trainium2, 8 NeuronCores.

Strategy:
  - Shard batch dim: 16 batches -> 2 per core; each core handles 128
    (batch, channel) lanes x 65536 samples.
  - All 3 levels are fused into one pass: each output sample of
    lo3/hi3/hi2/hi1 is a direct FIR of x (composite filters, 50 taps max,
    circular). Per 128-sample block of x, 128 outputs (64 hi1 + 32 hi2 +
    16 hi3 + 16 lo3) are a banded matrix multiply of the block plus a
    right-side halo (one-sided via output index shift; the host un-shifts
    with np.roll during unsharding).
  - On-chip pipeline per stripe of 32 blocks:
      DMA in (lane-major, contiguous) ->
      PE transpose 128x128 tiles (exact, fp32) -> position-major ->
      2 banded matmuls in fp32r (full speed at free dim 512) ->
      PE transpose back -> channel-grouped staging -> contiguous DMA out.
"""
import sys

sys.path.insert(0, "/opt/trn_rl_repo")

import numpy as np

import concourse.bass as bass
import concourse.bacc as bacc
import concourse.mybir as mybir
from concourse.tile import TileContext
from concourse import bass_utils

F32 = mybir.dt.float32
F32R = mybir.dt.float32r

# Problem constants (hardcoded per harness contract)
B, C, N = 16, 64, 65536
NCORES = 8
LANES = 128            # (B/NCORES) * C = 2*64 lanes per core
NBLK = N // 128        # 512 blocks per lane
SB = 32                # blocks per stripe
NSTRIPE = NBLK // SB   # 16 stripes

TRACE = False
LAST_RESULT = None
_NC_CACHE = None


def _build_filters(h0, h1):
    h = h0.astype(np.float64)
    g = h1.astype(np.float64)
    L = 8
    f2h = np.zeros(22)
    f2g = np.zeros(22)
    for s in range(L):
        for t in range(L):
            f2h[2 * s + t] += h[s] * h[t]
            f2g[2 * s + t] += g[s] * h[t]
    f3h = np.zeros(50)
    f3g = np.zeros(50)
    for s in range(L):
        for u in range(22):
            f3h[4 * s + u] += h[s] * f2h[u]
            f3g[4 * s + u] += g[s] * f2h[u]
    return f2g, f3g, f3h


def build_W(h0, h1):
    """Wbig2[q, c] for q in [0, 256): coeff of x[128b + q] in channel c of
    block b, with shifted output indexing:
      c in [0,64):    hi1[64b + 2 + c]   q = 2c + 1 + t,  t in [0,8)
      c in [64,96):   hi2[32b + 3 + c']  q = 4c' + 3 + u, u in [0,22)
      c in [96,112):  hi3[16b + 3 + c']  q = 8c' + 3 + v, v in [0,50)
      c in [112,128): lo3[16b + 3 + c']  q = 8c' + 3 + v
    Host unshifts with np.roll(out, shift) where shift = 2 (hi1) or 3.
    """
    f2g, f3g, f3h = _build_filters(h0, h1)
    g = h1.astype(np.float64)
    Wbig = np.zeros((256, 128))
    for c in range(128):
        if c < 64:
            o, d, taps, stride = c, 1, g, 2
        elif c < 96:
            o, d, taps, stride = c - 64, 3, f2g, 4
        elif c < 112:
            o, d, taps, stride = c - 96, 3, f3g, 8
        else:
            o, d, taps, stride = c - 112, 3, f3h, 8
        for t in range(len(taps)):
            q = stride * o + d + t
            Wbig[q, c] += taps[t]
    return Wbig.astype(np.float32)


def build_nc():
    nc = bacc.Bacc(target_bir_lowering=False)
    x = nc.dram_tensor("x", [LANES, N], F32, kind="ExternalInput")
    # wmat: [128, 256] = [W0 | Wp]; Wp nonzero only in rows 0..44
    wmat = nc.dram_tensor("wmat", [128, 256], F32, kind="ExternalInput")
    ident = nc.dram_tensor("ident", [128, 128], F32, kind="ExternalInput")
    hi1 = nc.dram_tensor("hi1", [LANES, N // 2], F32, kind="ExternalOutput")
    hi2 = nc.dram_tensor("hi2", [LANES, N // 4], F32, kind="ExternalOutput")
    hi3 = nc.dram_tensor("hi3", [LANES, N // 8], F32, kind="ExternalOutput")
    lo3 = nc.dram_tensor("lo3", [LANES, N // 8], F32, kind="ExternalOutput")

    SBC = SB * 128           # stripe columns (conv region)
    XC = (SB + 1) * 128      # stripe + right halo block

    with TileContext(nc) as tc:
        with (
            tc.tile_pool(name="consts", bufs=1) as cpool,
            tc.tile_pool(name="xin", bufs=3) as xin_pool,
            tc.tile_pool(name="xt", bufs=2) as xt_pool,
            tc.tile_pool(name="conv", bufs=3) as conv_pool,
            tc.tile_pool(name="outp", bufs=2) as out_pool,
            tc.tile_pool(name="pst", bufs=2, space="PSUM") as pst_pool,
            tc.tile_pool(name="psc", bufs=2, space="PSUM") as psc_pool,
            tc.tile_pool(name="pso", bufs=2, space="PSUM") as pso_pool,
        ):
            wt = cpool.tile([128, 256], F32)
            wtr = cpool.tile([128, 256], F32R)
            it = cpool.tile([128, 128], F32)
            nc.sync.dma_start(wt[:], wmat[:])
            nc.sync.dma_start(it[:], ident[:])
            nc.vector.tensor_copy(wtr[:], wt[:])
            w0 = wtr[:, 0:128]
            wp = wtr[:, 128:256]

            ecnt = [0]

            def copy(o, i):
                if ecnt[0] % 2 == 0:
                    nc.vector.tensor_copy(o, i)
                else:
                    nc.scalar.copy(o, i)
                ecnt[0] += 1

            for s in range(NSTRIPE):
                b0 = s * SB
                x_sb = xin_pool.tile([128, XC], F32, name=f"xsb{s}", tag="xsb")
                # load blocks b0 .. b0+SB (right halo, circular at the end)
                if s == NSTRIPE - 1:
                    nc.sync.dma_start(
                        x_sb[:, 0:SBC], x[:, b0 * 128:(b0 + SB) * 128])
                    nc.sync.dma_start(x_sb[:, SBC:XC], x[:, 0:128])
                else:
                    nc.sync.dma_start(
                        x_sb[:], x[:, b0 * 128:(b0 + SB + 1) * 128])

                # T-in: transpose each 128x128 tile -> xt (position-major)
                xt = xt_pool.tile([128, XC], F32R, name=f"xt{s}", tag="xt")
                n_t = SB + 1
                for j0 in range(0, n_t, 4):
                    jn = min(4, n_t - j0)
                    ps = pst_pool.tile([128, 512], F32, name=f"ps{s}_{j0}",
                                       tag="pst")
                    for j in range(j0, j0 + jn):
                        nc.tensor.matmul(
                            ps[:, (j - j0) * 128:(j - j0 + 1) * 128],
                            x_sb[:, j * 128:(j + 1) * 128],
                            it[:],
                            is_transpose=True,
                            start=(j == j0),
                            stop=(j == j0 + jn - 1),
                        )
                    copy(xt[:, j0 * 128:(j0 + jn) * 128], ps[:, 0:jn * 128])

                # conv: 2 banded matmuls (cur + right halo), fp32r
                conv_sb = conv_pool.tile([128, SBC], F32, name=f"cv{s}",
                                         tag="conv")
                for gi in range(SBC // 512):
                    pc = psc_pool.tile([128, 512], F32, name=f"pc{s}_{gi}",
                                       tag="psc")
                    base = gi * 512
                    nc.tensor.matmul(
                        pc[:], w0, xt[:, base:base + 512],
                        start=True, stop=False,
                    )
                    nc.tensor.matmul(
                        pc[:], wp, xt[:, base + 128:base + 640],
                        start=False, stop=True,
                    )
                    copy(conv_sb[:, gi * 512:(gi + 1) * 512], pc[:])

                # T-out: transpose back to lane-major, 8 blocks per 2-bank
                # psum tile, then channel-grouped copies into staging tiles
                h1_sb = out_pool.tile([128, SB * 64], F32, name=f"h1s{s}",
                                      tag="h1s")
                h2_sb = out_pool.tile([128, SB * 32], F32, name=f"h2s{s}",
                                      tag="h2s")
                h3_sb = out_pool.tile([128, SB * 16], F32, name=f"h3s{s}",
                                      tag="h3s")
                l3_sb = out_pool.tile([128, SB * 16], F32, name=f"l3s{s}",
                                      tag="l3s")
                for j0 in range(0, SB, 8):
                    po = pso_pool.tile([128, 1024], F32, name=f"po{s}_{j0}",
                                       tag="pso")
                    for j in range(j0, j0 + 8):
                        nc.tensor.matmul(
                            po[:, (j - j0) * 128:(j - j0 + 1) * 128],
                            conv_sb[:, j * 128:(j + 1) * 128],
                            it[:],
                            is_transpose=True,
                            start=((j - j0) % 4 == 0),
                            stop=((j - j0) % 4 == 3),
                        )
                    pv = po[:].rearrange("p (j c) -> p j c", c=128)
                    copy(
                        h1_sb[:, j0 * 64:(j0 + 8) * 64].rearrange(
                            "p (j c) -> p j c", c=64),
                        pv[:, :, 0:64])
                    copy(
                        h2_sb[:, j0 * 32:(j0 + 8) * 32].rearrange(
                            "p (j c) -> p j c", c=32),
                        pv[:, :, 64:96])
                    copy(
                        h3_sb[:, j0 * 16:(j0 + 8) * 16].rearrange(
                            "p (j c) -> p j c", c=16),
                        pv[:, :, 96:112])
                    copy(
                        l3_sb[:, j0 * 16:(j0 + 8) * 16].rearrange(
                            "p (j c) -> p j c", c=16),
                        pv[:, :, 112:128])

                # DMA out: fully contiguous on both sides
                nc.sync.dma_start(hi1[:, 64 * b0:64 * (b0 + SB)], h1_sb[:])
                nc.sync.dma_start(hi2[:, 32 * b0:32 * (b0 + SB)], h2_sb[:])
                nc.sync.dma_start(hi3[:, 16 * b0:16 * (b0 + SB)], h3_sb[:])
                nc.sync.dma_start(lo3[:, 16 * b0:16 * (b0 + SB)], l3_sb[:])
    nc.finalize()
    return nc


def _get_nc():
    global _NC_CACHE
    if _NC_CACHE is None:
        _NC_CACHE = build_nc()
    return _NC_CACHE


def kernel(x, h0, h1, **_ignored):
    x = np.ascontiguousarray(np.asarray(x, dtype=np.float32))
    h0 = np.asarray(h0, dtype=np.float32)
    h1 = np.asarray(h1, dtype=np.float32)
    assert x.shape == (B, C, N)

    Wbig = build_W(h0, h1)  # (256, 128)
    wmat = np.ascontiguousarray(
        np.concatenate([Wbig[0:128], Wbig[128:256]], axis=1),
        dtype=np.float32)
    ident = np.eye(128, dtype=np.float32)

    nc = _get_nc()
    bper = B // NCORES
    in_maps = []
    for i in range(NCORES):
        xs = x[i * bper:(i + 1) * bper].reshape(LANES, N)
        in_maps.append({
            "x": np.ascontiguousarray(xs),
            "wmat": wmat,
            "ident": ident,
        })
    kwargs = {}
    if TRACE:
        kwargs = dict(trace=True, trace_cores=[0])
    res = bass_utils.run_bass_kernel_spmd(
        nc, in_maps, core_ids=list(range(NCORES)), **kwargs
    )
    global LAST_RESULT
    LAST_RESULT = res

    def gather(name, n_out, shift):
        full = np.concatenate(
            [r[name].reshape(bper, C, n_out) for r in res.results], axis=0)
        return np.roll(full, shift, axis=-1)

    lo3 = gather("lo3", N // 8, 3)
    hi1 = gather("hi1", N // 2, 2)
    hi2 = gather("hi2", N // 4, 3)
    hi3 = gather("hi3", N // 8, 3)
    return lo3, hi1, hi2, hi3
